# revision 2
# baseline (speedup 1.0000x reference)
"""Positional-encoding add kernel for Trainium2 (8 NeuronCores, SPMD).

Problem: X[4, 4096, 2048] f32; out = X + PE[None, :, :] where
  PE[s, 2i]   = sin(s / 10000^(2i/2048))
  PE[s, 2i+1] = cos(s / 10000^(2i/2048))

Sharding: sequence dim split 8 ways -> 512 positions per core.
Per core the shard is [4, 512, 2048] = 16 MiB, flattened to rows
[2048, 2048] (row = b*512 + s_local).

Design notes (HW-measured; see memory/trn2-pe-kernel-timing.md):
- Per-NC HBM limit ~358 GB/s one direction at a time (716 GB/s per
  stack shared by 2 NCs); mixing directions degrades the aggregate.
  So the kernel is phased: load all 16 MiB (reads only), add in
  place, then store (writes only). Phase separation falls out of the
  single HWDGE ring's FIFO order - stores enqueue behind all loads.
- The PE table is generated on device. Only a 4 KiB invf row + 2 KiB
  positions come from HBM: the [1,1024] row is partition-broadcast
  on-chip to [128,1024] (this replaced a 512 KiB broadcast table in
  HBM - the one measurable HBM saving left over the staged baseline).
    a  = invf * pos                      (DVE, per-partition scalar)
    k  = round(a / 2pi)  [+0.25 for cos] (ACT Identity with i32 out)
    m  = a - 2pi*k  in [-pi, pi]         (DVE scalar_tensor_tensor)
    pe = Sin(m)  /  Sin(m2 + pi/2)       (ACT; Sin accurate in-range)
  End-to-end rel err 1.85e-05 (tolerance 2e-2).
- Measured phased floor: load-only ~47 us, store-only ~45 us per
  16.78 MB; phased load+store body ~99-101 us/rep incl. the direction
  turnaround. Overlapped/duplex variants measured worse (direction
  mixing); DMA granularity 1 MiB vs 4 MiB measured equal within noise.
"""

import os

import numpy as np

B, S, D = 4, 4096, 2048
N_CORES = 8
S_SHARD = S // N_CORES          # 512 positions per core
ROWS = B * S_SHARD              # 2048 rows per core
P = 128                         # SBUF partitions
K = D // 2                      # 1024 frequencies
N_TILES = ROWS // P             # 16 x [128, 2048] 1 MiB tiles
N_PE = S_SHARD // P             # 4 PE blocks [128, 2048]

_cached_nc = None
LAST_RESULT = None              # BassKernelResults of the last run (for test.py)


def _build_nc(repeat: int = 1):
    import concourse.bacc as bacc
    import concourse.mybir as mybir
    from concourse.tile import TileContext

    f32 = mybir.dt.float32
    i32 = mybir.dt.int32
    PI = float(np.pi)
    Sin = mybir.ActivationFunctionType.Sin
    Ident = mybir.ActivationFunctionType.Identity
    A = mybir.AluOpType

    nc = bacc.Bacc(None, target_bir_lowering=False, debug=False)
    x = nc.dram_tensor("X", [ROWS, D], f32, kind="ExternalInput")
    invf1 = nc.dram_tensor("INVF1", [1, K], f32, kind="ExternalInput")
    pos = nc.dram_tensor("POS", [P, N_PE], f32, kind="ExternalInput")
    out = nc.dram_tensor("OUT", [ROWS, D], f32, kind="ExternalOutput")

    # tile t covers rows [t*128, (t+1)*128); row = 512*b + s_local,
    # so tile t is batch t//4, position block (t%4)*128 -> PE block t%4.
    xv = x.rearrange("(t p) d -> t p d", t=N_TILES, p=P)
    ov = out.rearrange("(t p) d -> t p d", t=N_TILES, p=P)

    with TileContext(nc) as tc:
        with (
            tc.tile_pool(name="pe", bufs=1) as pe_pool,
            tc.tile_pool(name="xs", bufs=1) as xs_pool,
            tc.tile_pool(name="gen", bufs=1) as gen_pool,
        ):
            pe_ts = [
                pe_pool.tile([P, D], f32, name=f"pe{t}") for t in range(N_PE)
            ]
            x_ts = [
                xs_pool.tile([P, D], f32, name=f"x{t}") for t in range(N_TILES)
            ]
            invf_row = gen_pool.tile([1, K], f32, name="invf_row")
            invf_t = gen_pool.tile([P, K], f32, name="invf_t")
            pos_t = gen_pool.tile([P, N_PE], f32, name="pos_t")
            pi2 = gen_pool.tile([P, 1], f32, name="pi2")
            qtr = gen_pool.tile([P, 1], f32, name="qtr")
            a_ts = [gen_pool.tile([P, K], f32, name=f"a{b}") for b in range(2)]
            m_ts = [gen_pool.tile([P, K], f32, name=f"m{b}") for b in range(2)]
            k1_t = gen_pool.tile([P, K], i32, name="k1_t")
            k2_t = gen_pool.tile([P, K], i32, name="k2_t")

            # tiny PE-gen inputs on the ACT ring; X loads own the sync ring
            nc.scalar.dma_start(out=invf_row, in_=invf1[:, :])
            nc.scalar.dma_start(out=pos_t, in_=pos[:, :])
            nc.gpsimd.partition_broadcast(invf_t[:, :], invf_row[:, :])
            nc.vector.memset(pi2, PI / 2)
            nc.vector.memset(qtr, 0.25)

            # on-device PE: block b holds positions pos_t[:, b]
            for b in range(N_PE):
                a = a_ts[b % 2]
                m2 = m_ts[b % 2]
                nc.vector.tensor_scalar(
                    out=a, in0=invf_t, scalar1=pos_t[:, b : b + 1],
                    scalar2=None, op0=A.mult)
                nc.scalar.activation(out=k1_t, in_=a, func=Ident,
                                     scale=float(1 / (2 * PI)))
                nc.scalar.activation(out=k2_t, in_=a, func=Ident,
                                     scale=float(1 / (2 * PI)),
                                     bias=qtr[:, 0:1])
                nc.vector.scalar_tensor_tensor(
                    out=m2, in0=k2_t, scalar=float(-2 * PI), in1=a,
                    op0=A.mult, op1=A.add)
                nc.scalar.activation(out=pe_ts[b][:, 1::2], in_=m2, func=Sin,
                                     bias=pi2[:, 0:1])
                nc.vector.scalar_tensor_tensor(
                    out=a, in0=k1_t, scalar=float(-2 * PI), in1=a,
                    op0=A.mult, op1=A.add)
                nc.scalar.activation(out=pe_ts[b][:, 0::2], in_=a, func=Sin)

            for _rep in range(repeat):
                for t in range(N_TILES):
                    nc.sync.dma_start(out=x_ts[t], in_=xv[t])
                for t in range(N_TILES):
                    nc.vector.tensor_add(
                        out=x_ts[t], in0=x_ts[t], in1=pe_ts[t % N_PE]
                    )
                # stores enqueue on the same ring behind all loads: the
                # FIFO keeps HBM one-directional in each phase.
                for t in range(N_TILES):
                    nc.sync.dma_start(out=ov[t], in_=x_ts[t])
    nc.finalize()
    return nc


def _invf_row() -> np.ndarray:
    """invf[k] = 1 / 10000^(2k/D) as f32, matching the jax f32 reference."""
    try:
        import jax

        with jax.default_device(jax.devices("cpu")[0]):
            import jax.numpy as jnp

            i = jnp.arange(K, dtype=jnp.float32)[None, :]
            denom = jnp.power(jnp.asarray(10000.0, jnp.float32), 2.0 * i / D)
            return np.asarray(1.0 / denom, dtype=np.float32).reshape(K)
    except Exception:
        i = np.arange(K, dtype=np.float32)
        expo = ((np.float32(2.0) * i) / np.float32(D)).astype(np.float32)
        denom = np.power(np.float32(10000.0), expo, dtype=np.float32)
        return (np.float32(1.0) / denom).astype(np.float32)


def _in_maps(X: np.ndarray) -> list:
    """Per-core input dicts for the SPMD kernel."""
    invf1 = np.ascontiguousarray(_invf_row().reshape(1, K), dtype=np.float32)
    in_maps = []
    for c in range(N_CORES):
        xs = np.ascontiguousarray(
            X[:, c * S_SHARD : (c + 1) * S_SHARD, :]
        ).reshape(ROWS, D)
        p = np.arange(P, dtype=np.float32)[:, None]
        b = np.arange(N_PE, dtype=np.float32)[None, :]
        posc = (c * S_SHARD + b * P + p).astype(np.float32)
        in_maps.append({"X": xs, "INVF1": invf1, "POS": posc})
    return in_maps


def kernel(X: np.ndarray) -> np.ndarray:
    global _cached_nc, LAST_RESULT
    from concourse.bass_utils import run_bass_kernel_spmd

    X = np.asarray(X)
    assert X.shape == (B, S, D), X.shape
    X = np.ascontiguousarray(X, dtype=np.float32)

    if _cached_nc is None:
        _cached_nc = _build_nc()
    nc = _cached_nc

    trace = bool(int(os.environ.get("KERNEL_TRACE", "0")))
    res = run_bass_kernel_spmd(
        nc, _in_maps(X), core_ids=list(range(N_CORES)), trace=trace
    )
    LAST_RESULT = res

    out = np.empty((B, S, D), dtype=np.float32)
    for c in range(N_CORES):
        out[:, c * S_SHARD : (c + 1) * S_SHARD, :] = res.results[c]["OUT"].reshape(
            B, S_SHARD, D
        )
    return out


# revision 3
# speedup vs baseline: 1.0326x; 1.0326x over previous
"""Positional-encoding add kernel for Trainium2 (8 NeuronCores, SPMD).

Problem: X[4, 4096, 2048] f32; out = X + PE[None, :, :] where
  PE[s, 2i]   = sin(s / 10000^(2i/2048))
  PE[s, 2i+1] = cos(s / 10000^(2i/2048))

Sharding: sequence dim split 8 ways -> 512 positions per core; shard
[4, 512, 2048] = 16 MiB flattened to rows [2048, 2048] (row = 512b + s).

4 MiB DMA granularity: supertile c = batch c. Partition p holds DRAM
rows 512c+4p .. 512c+4p+3 (32 KB contiguous per partition), i.e.
position s = 4p + j, j in 0..3. One PE supertile [128, 4*2048] with the
same s = 4p + j layout matches EVERY batch supertile, so each rep is
just 4 loads + 4 adds + 4 stores. Grouped A/B measured 4 MiB DMAs
~2.7 us/rep faster than 16x1MiB (fewer ring entries, 32 KB vs 8 KB
contiguous descriptors).

Phasing (HW-measured): per-NC HBM is ~358 GB/s one direction at a
time; mixing directions degrades the aggregate. All X loads and OUT
stores sit on the single sync HWDGE ring - FIFO order keeps HBM
one-directional per phase with stores behind all loads. Duplex and
2-ring splits measured equal or worse.

PE is generated on device; only a 4 KiB invf row ([1,1024], partition-
broadcast on chip) + 2 KiB positions come from HBM:
    a  = invf * pos                      (DVE, per-partition scalar)
    k  = round(a / 2pi)  [+0.25 for cos] (ACT Identity with i32 out)
    m  = a - 2pi*k  in [-pi, pi]         (DVE scalar_tensor_tensor)
    pe = Sin(m)  /  Sin(m2 + pi/2)       (ACT; Sin accurate in-range)
invf comes from host f32 pow, matching the jax reference bit-for-bit.
"""

import os

import numpy as np

B, S, D = 4, 4096, 2048
N_CORES = 8
S_SHARD = S // N_CORES          # 512 positions per core
ROWS = B * S_SHARD              # 2048 rows per core
P = 128                         # SBUF partitions
K = D // 2                      # 1024 frequencies
J = S_SHARD // P                # 4 positions per partition (s = 4p + j)

_cached_nc = None
LAST_RESULT = None              # BassKernelResults of the last run (for test.py)


def _build_nc(repeat: int = 1):
    import concourse.bacc as bacc
    import concourse.mybir as mybir
    from concourse.tile import TileContext

    f32 = mybir.dt.float32
    i32 = mybir.dt.int32
    PI = float(np.pi)
    Sin = mybir.ActivationFunctionType.Sin
    Ident = mybir.ActivationFunctionType.Identity
    A = mybir.AluOpType

    nc = bacc.Bacc(None, target_bir_lowering=False, debug=False)
    x = nc.dram_tensor("X", [ROWS, D], f32, kind="ExternalInput")
    invf1 = nc.dram_tensor("INVF1", [1, K], f32, kind="ExternalInput")
    pos = nc.dram_tensor("POS", [P, J], f32, kind="ExternalInput")
    out = nc.dram_tensor("OUT", [ROWS, D], f32, kind="ExternalOutput")

    xv = x.rearrange("(c p j) d -> c p (j d)", c=B, p=P, j=J)
    ov = out.rearrange("(c p j) d -> c p (j d)", c=B, p=P, j=J)

    with TileContext(nc) as tc:
        with (
            tc.tile_pool(name="pe", bufs=1) as pe_pool,
            tc.tile_pool(name="xs", bufs=1) as xs_pool,
            tc.tile_pool(name="gen", bufs=1) as gen_pool,
        ):
            pe_t = pe_pool.tile([P, J * D], f32, name="pe")
            x_ts = [xs_pool.tile([P, J * D], f32, name=f"x{c}") for c in range(B)]
            invf_row = gen_pool.tile([1, K], f32, name="invf_row")
            invf_t = gen_pool.tile([P, K], f32, name="invf_t")
            pos_t = gen_pool.tile([P, J], f32, name="pos_t")
            pi2 = gen_pool.tile([P, 1], f32, name="pi2")
            qtr = gen_pool.tile([P, 1], f32, name="qtr")
            a_ts = [gen_pool.tile([P, K], f32, name=f"a{j}") for j in range(2)]
            m_ts = [gen_pool.tile([P, K], f32, name=f"m{j}") for j in range(2)]
            k1_t = gen_pool.tile([P, K], i32, name="k1_t")
            k2_t = gen_pool.tile([P, K], i32, name="k2_t")

            # tiny PE-gen inputs on the ACT ring; X loads own the sync ring
            nc.scalar.dma_start(out=invf_row, in_=invf1[:, :])
            nc.scalar.dma_start(out=pos_t, in_=pos[:, :])
            nc.gpsimd.partition_broadcast(invf_t[:, :], invf_row[:, :])
            nc.vector.memset(pi2, PI / 2)
            nc.vector.memset(qtr, 0.25)

            # on-device PE: column block j holds positions pos_t[:, j] = 4p+j
            for j in range(J):
                a = a_ts[j % 2]
                m2 = m_ts[j % 2]
                nc.vector.tensor_scalar(
                    out=a, in0=invf_t, scalar1=pos_t[:, j : j + 1],
                    scalar2=None, op0=A.mult)
                nc.scalar.activation(out=k1_t, in_=a, func=Ident,
                                     scale=float(1 / (2 * PI)))
                nc.scalar.activation(out=k2_t, in_=a, func=Ident,
                                     scale=float(1 / (2 * PI)),
                                     bias=qtr[:, 0:1])
                nc.vector.scalar_tensor_tensor(
                    out=m2, in0=k2_t, scalar=float(-2 * PI), in1=a,
                    op0=A.mult, op1=A.add)
                nc.scalar.activation(
                    out=pe_t[:, j * D + 1 : (j + 1) * D : 2], in_=m2,
                    func=Sin, bias=pi2[:, 0:1])
                nc.vector.scalar_tensor_tensor(
                    out=a, in0=k1_t, scalar=float(-2 * PI), in1=a,
                    op0=A.mult, op1=A.add)
                nc.scalar.activation(
                    out=pe_t[:, j * D : (j + 1) * D : 2], in_=a, func=Sin)

            for _rep in range(repeat):
                for c in range(B):
                    nc.sync.dma_start(out=x_ts[c], in_=xv[c])
                for c in range(B):
                    nc.vector.tensor_add(out=x_ts[c], in0=x_ts[c], in1=pe_t)
                # stores enqueue on the same ring behind all loads: the
                # FIFO keeps HBM one-directional in each phase.
                for c in range(B):
                    nc.sync.dma_start(out=ov[c], in_=x_ts[c])
    nc.finalize()
    return nc


def _invf_row() -> np.ndarray:
    """invf[k] = 1 / 10000^(2k/D) as f32, matching the jax f32 reference."""
    try:
        import jax

        with jax.default_device(jax.devices("cpu")[0]):
            import jax.numpy as jnp

            i = jnp.arange(K, dtype=jnp.float32)[None, :]
            denom = jnp.power(jnp.asarray(10000.0, jnp.float32), 2.0 * i / D)
            return np.asarray(1.0 / denom, dtype=np.float32).reshape(K)
    except Exception:
        i = np.arange(K, dtype=np.float32)
        expo = ((np.float32(2.0) * i) / np.float32(D)).astype(np.float32)
        denom = np.power(np.float32(10000.0), expo, dtype=np.float32)
        return (np.float32(1.0) / denom).astype(np.float32)


def _in_maps(X: np.ndarray) -> list:
    """Per-core input dicts for the SPMD kernel."""
    invf1 = np.ascontiguousarray(_invf_row().reshape(1, K), dtype=np.float32)
    in_maps = []
    for c in range(N_CORES):
        xs = np.ascontiguousarray(
            X[:, c * S_SHARD : (c + 1) * S_SHARD, :]
        ).reshape(ROWS, D)
        p = np.arange(P, dtype=np.float32)[:, None]
        j = np.arange(J, dtype=np.float32)[None, :]
        posc = (c * S_SHARD + J * p + j).astype(np.float32)
        in_maps.append({"X": xs, "INVF1": invf1, "POS": posc})
    return in_maps


def kernel(X: np.ndarray) -> np.ndarray:
    global _cached_nc, LAST_RESULT
    from concourse.bass_utils import run_bass_kernel_spmd

    X = np.asarray(X)
    assert X.shape == (B, S, D), X.shape
    X = np.ascontiguousarray(X, dtype=np.float32)

    if _cached_nc is None:
        _cached_nc = _build_nc()
    nc = _cached_nc

    trace = bool(int(os.environ.get("KERNEL_TRACE", "0")))
    res = run_bass_kernel_spmd(
        nc, _in_maps(X), core_ids=list(range(N_CORES)), trace=trace
    )
    LAST_RESULT = res

    out = np.empty((B, S, D), dtype=np.float32)
    for c in range(N_CORES):
        out[:, c * S_SHARD : (c + 1) * S_SHARD, :] = res.results[c]["OUT"].reshape(
            B, S_SHARD, D
        )
    return out
